# revision 58
# baseline (speedup 1.0000x reference)
"""Multi-head attention (B=2, S=2048, D=1024, H=16) on 8 TRN2 NeuronCores.

Sharding: core c handles batch c//4 and head-group c%4 (4 heads each).
Host pre-transposes inputs/weights to d-major bf16; each core computes its
4 heads' projections, causal attention, and a partial (row-parallel) dense
output [S, D] which the host sums across the 4 cores of each batch.

v2 rewrite (baseline 197.5us):
- j-outer emission with an explicit PE fill queue: projection/dense psum
  blocks are interleaved between attention score groups so the in-order PE
  never stalls behind the ScalarE exp chain (the ~80us softmax floor).
- exp is emitted as two [128,2,512] halves per 4-tile group so the
  single-buffered scores psum ring (4 banks) can restart scores for the
  next group after only half the exp has drained.
- softmax denominator: odd heads place V at lhsT cols 64:128 (av lands at
  psum partitions 64:128 directly - no partition bounce) and the ones
  column at col 0 (den at partition 0, where the fast reciprocal runs).
  Partition-broadcast of the denominator/reciprocal runs on the PE as a
  K=128 matmul against a one-hot-row ones matrix (M padded to 128 so every
  matmul in the kernel keeps tile_size (128,128) - no PE mode switches).
  Zero DMAs in the whole normalization chain (baseline used 3 per head).
- dense psum evacuation split DVE/GpSimd; ScalarE does exp only.
"""

import numpy as np
import ml_dtypes
from contextlib import ExitStack

import concourse.bass as bass
import concourse.tile as tile
from concourse import bacc, mybir
from concourse.bass_utils import run_bass_kernel_spmd

BF16 = mybir.dt.bfloat16
F32 = mybir.dt.float32
NPBF16 = ml_dtypes.bfloat16

D_MODEL = 1024
NH = 16
DEPTH = 64
B = 2
S = 2048
N_CORES = 8
GROUPS = 4              # head-groups (tensor parallel dimension)
HPG = NH // GROUPS      # 4 heads per core
OG = HPG * DEPTH        # 256 projection output cols per core
QC = 512                # q chunk (matmul free dim)
NQC = S // QC           # 4
KT = 128                # k tile (psum partition dim)
NKT = S // KT           # 16
DK = D_MODEL // 128     # 8 contraction tiles of 128
SC = 512                # projection s chunk
NSC = S // SC           # 4
EGRP = 2                # k-tiles per exp group (psum group tile)

TRACE = False
TRACE_KW = {}
LAST_RESULT = None
DEBUG = False
_CACHE = {}


def _chunk(lst, n):
    return [lst[i : i + n] for i in range(0, len(lst), n)]


def _build(ktiles, n_uniq, zero_bias):
    """Emit the bass program. ktiles[j] = [(t, lo, tri), ...] computed
    k-tiles for q-chunk j (see _classify_mask)."""
    nc = bacc.Bacc(
        "TRN2", target_bir_lowering=False, debug=False, num_devices=N_CORES
    )
    # inputs pre-split into contiguous S-quarters for fat DMA descriptors
    xq = nc.dram_tensor("xq", [NSC, 128, DK, SC], BF16, kind="ExternalInput").ap()
    xk = nc.dram_tensor("xk", [NSC, 128, DK, SC], BF16, kind="ExternalInput").ap()
    xv = nc.dram_tensor("xv", [NSC, 128, DK, SC], BF16, kind="ExternalInput").ap()
    wq = nc.dram_tensor("wq", [128, DK, OG], BF16, kind="ExternalInput").ap()
    wk = nc.dram_tensor("wk", [128, DK, OG], BF16, kind="ExternalInput").ap()
    wv = nc.dram_tensor("wv", [128, DK, OG], BF16, kind="ExternalInput").ap()
    wd = nc.dram_tensor("wd", [128, 2, D_MODEL], BF16, kind="ExternalInput").ap()
    qb = nc.dram_tensor("qb", [128, 2], F32, kind="ExternalInput").ap()
    kb = nc.dram_tensor("kb", [128, 2], F32, kind="ExternalInput").ap()
    mk = nc.dram_tensor("mk", [128, n_uniq, KT], BF16, kind="ExternalInput").ap()
    # bf16 partial outputs: host sums the 4 row-parallel partials in f32
    outp = nc.dram_tensor("outp", [S, D_MODEL], BF16, kind="ExternalOutput").ap()

    Exp = mybir.ActivationFunctionType.Exp

    with tile.TileContext(nc) as tc, ExitStack() as ctx:
        singles = ctx.enter_context(tc.tile_pool(name="singles", bufs=1))
        exps = ctx.enter_context(tc.tile_pool(name="exps", bufs=4))
        small = ctx.enter_context(tc.tile_pool(name="small", bufs=2))
        # psum: scores ring 4 banks + proj/dense 2 + av accum 1 + bcast 1
        psc = ctx.enter_context(tc.tile_pool(name="psc", bufs=1, space="PSUM"))
        po = ctx.enter_context(tc.tile_pool(name="po", bufs=1, space="PSUM"))
        ost = ctx.enter_context(tc.tile_pool(name="ost", bufs=6))

        wq_sb = singles.tile([128, DK, OG], BF16)
        wk_sb = singles.tile([128, DK, OG], BF16)
        wv_sb = singles.tile([128, DK, OG], BF16)
        mk_sb = singles.tile([128, n_uniq, KT], BF16)
        qb_sb = singles.tile([128, 2], F32)
        kb_sb = singles.tile([128, 2], F32)
        wd_sb = singles.tile([128, 2, D_MODEL], BF16)  # loaded late

        # per-head layouts, zero-padded to K=128 so every matmul keeps the
        # full (128,128) PE tile config (no mode-switch drains).
        # head h occupies d-rows [(h%2)*64, (h%2)*64+64); the rest are zeros.
        qt = singles.tile([128, HPG, S], BF16)
        kt_ = singles.tile([128, HPG, S], BF16)
        # [p=k%128, ktile, head, 128 cols]
        # even heads: V d at cols 0:64, ones col 64 (lhsT slice [:, :65])
        # odd heads:  ones col 0, zeros 1:64, V d at cols 64:128
        vh1 = singles.tile([128, NKT, HPG, 128], BF16)
        avb = singles.tile([128, 2, S], BF16)   # normalized av^T (pair layout)
        # one-hot-row broadcast weights: slot 0 row 64 (even den), slot 1
        # row 0 (odd rec); M padded to 128 with zero cols to stay (128,128)
        ones_bc = singles.tile([128, 2, 128], BF16)
        # staging row for the den/rec value fed to the broadcast matmul
        den_sb = singles.tile([128, 2, QC], BF16)

        # 0/1 row masks: the projection evacuations below write each padded
        # qt/kt head slot in ONE full-partition op (data rows x1, padding
        # rows x0) - no separate padding memsets on the critical path
        hmask = singles.tile([128, 2], F32)

        # setup memsets (gpsimd; overlap the initial DMAs)
        nc.vector.memset(hmask[0:64, 0:1], 1.0)
        nc.vector.memset(hmask[64:128, 0:1], 0.0)
        nc.vector.memset(hmask[0:64, 1:2], 0.0)
        nc.vector.memset(hmask[64:128, 1:2], 1.0)
        nc.gpsimd.memset(ones_bc[:], 0.0)
        nc.gpsimd.memset(ones_bc[64:65, 0, 0:64], 1.0)
        nc.gpsimd.memset(ones_bc[0:1, 1, 64:128], 1.0)
        nc.gpsimd.memset(den_sb[:], 0.0)
        nc.gpsimd.memset(vh1[:, :, 0::2, 64:65], 1.0)  # even ones col
        nc.gpsimd.memset(vh1[:, :, 1::2, 0:64], 0.0)   # odd ones col + zeros
        nc.gpsimd.memset(vh1[:, :, 1::2, 0:1], 1.0)
        nc.gpsimd.memset(vh1[:, :, 0::2, 65:128], 0.0)  # unread; keep sim clean

        def emit_proj_qk_oc(x_sb, w_sb, b_sb, dst, scn, oc):
            ssl = slice(scn * SC, (scn + 1) * SC)
            ps = po.tile([128, SC], F32, tag="ppav", bufs=2)
            for dk in range(DK):
                nc.tensor.matmul(
                    ps[:],
                    lhsT=w_sb[:, dk, oc * 128 : (oc + 1) * 128],
                    rhs=x_sb[:, dk, :],
                    start=(dk == 0),
                    stop=(dk == DK - 1),
                )
            if zero_bias:
                nc.vector.tensor_scalar(
                    out=dst[:, 2 * oc, ssl],
                    in0=ps[:],
                    scalar1=hmask[:, 0:1],
                    scalar2=None,
                    op0=mybir.AluOpType.mult,
                )
                nc.vector.tensor_scalar(
                    out=dst[:, 2 * oc + 1, ssl],
                    in0=ps[:],
                    scalar1=hmask[:, 1:2],
                    scalar2=None,
                    op0=mybir.AluOpType.mult,
                )
            else:
                nc.vector.tensor_scalar(
                    out=dst[:, 2 * oc, ssl],
                    in0=ps[:],
                    scalar1=b_sb[:, oc : oc + 1],
                    scalar2=hmask[:, 0:1],
                    op0=mybir.AluOpType.add,
                    op1=mybir.AluOpType.mult,
                )
                nc.vector.tensor_scalar(
                    out=dst[:, 2 * oc + 1, ssl],
                    in0=ps[:],
                    scalar1=b_sb[:, oc : oc + 1],
                    scalar2=hmask[:, 1:2],
                    op0=mybir.AluOpType.add,
                    op1=mybir.AluOpType.mult,
                )

        def emit_proj_v_st(xv_sb, scn, sth):
            st = scn * (SC // KT) + sth
            ps = po.tile([128, SC], F32, tag="ppav", bufs=2)
            for dk in range(DK):
                nc.tensor.matmul(
                    ps[:, :OG],
                    lhsT=xv_sb[:, dk, sth * KT : (sth + 1) * KT],
                    rhs=wv_sb[:, dk, :],
                    start=(dk == 0),
                    stop=(dk == DK - 1),
                )
            psh = ps[:, :OG].rearrange("p (h d) -> p h d", d=DEPTH)
            nc.vector.tensor_copy(out=vh1[:, st, 0::2, 0:64], in_=psh[:, 0::2, :])
            nc.vector.tensor_copy(out=vh1[:, st, 1::2, 64:128], in_=psh[:, 1::2, :])

        dense_ots = {}

        def emit_dense_oc(st, oc):
            # half a dense block: fine-grained fill unit (~0.9us PE)
            if oc == 0:
                dense_ots[st] = ost.tile(
                    [128, D_MODEL], BF16, tag="ostage", name="ot"
                )
            ot = dense_ots[st]
            ps = po.tile([128, SC], F32, tag="ppav", bufs=2)
            for co in range(2):
                nc.tensor.matmul(
                    ps[:],
                    lhsT=avb[:, co, st * 128 : (st + 1) * 128],
                    rhs=wd_sb[:, co, oc * 512 : (oc + 1) * 512],
                    start=(co == 0),
                    stop=(co == 1),
                )
            if st >= (NQC - 1) * 4 and oc == 1:
                # final-chunk blocks run in the kernel tail where the
                # Vector engine paces everything and ScalarE sits idle
                nc.scalar.copy(out=ot[:, oc * 512 : (oc + 1) * 512], in_=ps[:])
            else:
                nc.vector.tensor_copy(
                    out=ot[:, oc * 512 : (oc + 1) * 512], in_=ps[:]
                )
            if oc == 1:
                del dense_ots[st]
                nc.sync.dma_start(outp[st * 128 : (st + 1) * 128, :], ot[:])

        def emit_dense_st(st):
            emit_dense_oc(st, 0)
            emit_dense_oc(st, 1)

        def emit_dense_co0(st):
            # first contraction half (heads 0/1) of a last-chunk dense
            # block: runs as a fill once those heads' j=3 normalize is
            # done, pulling half of the final dense off the kernel tail
            dense_ots[st] = ost.tile([128, D_MODEL], BF16, tag="ostage", name="ot")
            ot = dense_ots[st]
            for oc in range(2):
                ps = po.tile([128, SC], F32, tag="ppav", bufs=2)
                nc.tensor.matmul(
                    ps[:],
                    lhsT=avb[:, 0, st * 128 : (st + 1) * 128],
                    rhs=wd_sb[:, 0, oc * 512 : (oc + 1) * 512],
                    start=True,
                    stop=True,
                )
                nc.vector.tensor_copy(
                    out=ot[:, oc * 512 : (oc + 1) * 512], in_=ps[:]
                )

        def emit_dense_co1(st):
            ot = dense_ots.pop(st)
            for oc in range(2):
                ps = po.tile([128, SC], F32, tag="ppav", bufs=2)
                nc.tensor.matmul(
                    ps[:],
                    lhsT=avb[:, 1, st * 128 : (st + 1) * 128],
                    rhs=wd_sb[:, 1, oc * 512 : (oc + 1) * 512],
                    start=True,
                    stop=True,
                )
                nc.vector.tensor_add(
                    ot[:, oc * 512 : (oc + 1) * 512],
                    ps[:],
                    ot[:, oc * 512 : (oc + 1) * 512],
                )
            nc.sync.dma_start(outp[st * 128 : (st + 1) * 128, :], ot[:])

        # ---- fill queues: deferred PE work interleaved between attention
        # groups so the in-order PE never stalls behind the exp chain.
        # hi (projections) must fully drain within the current j iteration
        # (next j's scores depend on them - a leftover would deadlock the
        # in-order PE); lo (dense) is rationed to keep the PE warm through
        # the fill-starved end of the last q-chunk. ----
        fills_hi = []
        fills_lo = []

        def pop_fill(lo_ok=True):
            if fills_hi:
                fills_hi.pop(0)()
            elif lo_ok and fills_lo:
                fills_lo.pop(0)()

        def emit_attn_head(h, j, prev_fin, horder):
            odd = h % 2
            ch = h // 2
            jsl = slice(j * QC, (j + 1) * QC)
            tiles = ktiles[j]
            first, last = tiles[0][0], tiles[-1][0]
            ngrp = (len(tiles) + EGRP - 1) // EGRP
            ps_av = po.tile([128, QC], F32, tag="psav", bufs=2)
            # G=2 groups through two alternating single-buffered psum tags:
            # scores of group g only wait on exp of group g-2 (true double
            # buffering) so the PE->ACT chain never serializes.
            for gi, grp in enumerate(_chunk(tiles, EGRP)):
                if gi == 1 and prev_fin is not None:
                    # previous head's den chain, deferred so its PE
                    # broadcast matmul hides behind this head's scores
                    prev_fin()
                    prev_fin = None
                ps_g = psc.tile(
                    [128, EGRP, QC], F32, tag=f"psc{gi % 2}", bufs=1
                )
                ex = exps.tile([128, EGRP, QC], BF16, tag="exps")
                for r, (t, lo, tri) in enumerate(grp):
                    nc.tensor.matmul(
                        ps_g[:, r, lo * 128 :],
                        lhsT=kt_[:, h, t * KT : (t + 1) * KT],
                        rhs=qt[:, h, j * QC + lo * 128 : (j + 1) * QC],
                        start=True,
                        stop=True,
                    )
                nc.scalar.activation(
                    out=ex[:, : len(grp), :],
                    in_=ps_g[:, : len(grp), :],
                    func=Exp,
                    scale=0.125,
                )
                for r, (t, lo, tri) in enumerate(grp):
                    for i, uid in tri:
                        nc.gpsimd.tensor_mul(
                            ex[:, r, i * 128 : (i + 1) * 128],
                            ex[:, r, i * 128 : (i + 1) * 128],
                            mk_sb[:, uid, :],
                        )
                for r, (t, lo, tri) in enumerate(grp):
                    if odd:
                        lhsT = vh1[:, t, h, :]        # den@p0, av@p64:128
                        o_sl = ps_av[:, lo * 128 :]
                    else:
                        lhsT = vh1[:, t, h, 0:65]     # av@p0:64, den@p64
                        o_sl = ps_av[0:65, lo * 128 :]
                    nc.tensor.matmul(
                        o_sl,
                        lhsT=lhsT,
                        rhs=ex[:, r, lo * 128 :],
                        start=(t == first),
                        stop=(t == last),
                    )
                # dense fills go preferentially to each head's last two
                # groups: the diagonal score tiles there are short and
                # leave the PE starved behind the exp chain. The final
                # head of the last chunk drains the co=0 dense fills.
                pop_fill(
                    lo_ok=gi >= ngrp - 2
                    or (j == NQC - 1 and h == horder[-1])
                )
            # stage the denominator row for the broadcast matmul right away
            # (DVE/gpsimd, off the PE stream); the PE part of the chain is
            # deferred into the next head's groups (see prev_fin above)
            if odd:
                rec0 = small.tile([1, QC], F32, tag="rec0")
                nc.vector.reciprocal_approx_fast(rec0[:], ps_av[0:1, :])
                nc.vector.tensor_copy(out=den_sb[0:1, 1, :], in_=rec0[:])
            else:
                nc.vector.tensor_copy(out=den_sb[64:65, 0, :], in_=ps_av[64:65, :])

            def fin():
                # normalization: reciprocal of the softmax denominator,
                # partition-broadcast via PE matmul, multiply out of psum.
                bc = po.tile([128, QC], F32, tag="ppav", bufs=2)
                if odd:
                    # rec row (partition 0) -> broadcast onto p64:128
                    nc.tensor.matmul(
                        bc[:], lhsT=ones_bc[:, 1, :], rhs=den_sb[:, 1, :],
                        start=True, stop=True,
                    )
                    rec_sb = small.tile([128, QC], F32, tag="recsb")
                    nc.vector.tensor_copy(
                        out=rec_sb[64:128, :], in_=bc[64:128, :]
                    )
                    nc.vector.tensor_mul(
                        avb[64:128, ch, jsl], ps_av[64:128, :], rec_sb[64:128, :]
                    )
                else:
                    # den row (partition 64) -> broadcast onto p0:64 ->
                    # recip (base partition 0) -> multiply
                    nc.tensor.matmul(
                        bc[:], lhsT=ones_bc[:, 0, :], rhs=den_sb[:, 0, :],
                        start=True, stop=True,
                    )
                    rec_sb = small.tile([128, QC], F32, tag="recsb")
                    nc.vector.reciprocal_approx_fast(rec_sb[0:64, :], bc[0:64, :])
                    nc.vector.tensor_mul(
                        avb[0:64, ch, jsl], ps_av[0:64, :], rec_sb[0:64, :]
                    )

            return fin

        # ---- emission: j-outer; proj chunk j+1 and dense j-1 become fill
        # units consumed inside attention j ----
        with tc.tile_pool(name="xin", bufs=6) as xin:
            x_sbs = {}

            def load_chunk(scn):
                for name, ap_ in (("xq", xq), ("xk", xk), ("xv", xv)):
                    t_ = xin.tile([128, DK, SC], BF16, tag="xin")
                    nc.sync.dma_start(t_[:], ap_[scn])
                    x_sbs[(name, scn)] = t_

            def proj_units(scn):
                u = []
                for oc in range(2):
                    u.append(lambda scn=scn, oc=oc: emit_proj_qk_oc(
                        x_sbs.pop(("xq", scn)) if oc == 1 else x_sbs[("xq", scn)],
                        wq_sb, qb_sb, qt, scn, oc))
                for oc in range(2):
                    u.append(lambda scn=scn, oc=oc: emit_proj_qk_oc(
                        x_sbs.pop(("xk", scn)) if oc == 1 else x_sbs[("xk", scn)],
                        wk_sb, kb_sb, kt_, scn, oc))
                for sth in range(SC // KT):
                    u.append(lambda scn=scn, sth=sth: emit_proj_v_st(
                        x_sbs.pop(("xv", scn)) if sth == SC // KT - 1
                        else x_sbs[("xv", scn)], scn, sth))
                return u

            # startup: DMAs ordered by first use (q weights + q chunk
            # first, in dk-halves so the first projection matmuls start
            # after half the transfer)
            nc.sync.dma_start(wq_sb[:, 0 : DK // 2], wq[:, 0 : DK // 2])
            t_ = xin.tile([128, DK, SC], BF16, tag="xin")
            nc.sync.dma_start(t_[:, 0 : DK // 2], xq[0, :, 0 : DK // 2])
            nc.sync.dma_start(wq_sb[:, DK // 2 :], wq[:, DK // 2 :])
            nc.sync.dma_start(t_[:, DK // 2 :], xq[0, :, DK // 2 :])
            x_sbs[("xq", 0)] = t_
            nc.sync.dma_start(wk_sb[:], wk)
            t_ = xin.tile([128, DK, SC], BF16, tag="xin")
            nc.sync.dma_start(t_[:], xk[0])
            x_sbs[("xk", 0)] = t_
            nc.sync.dma_start(wv_sb[:], wv)
            t_ = xin.tile([128, DK, SC], BF16, tag="xin")
            nc.sync.dma_start(t_[:], xv[0])
            x_sbs[("xv", 0)] = t_
            nc.sync.dma_start(mk_sb[:], mk)
            if not zero_bias:
                nc.sync.dma_start(qb_sb[:], qb)
                nc.sync.dma_start(kb_sb[:], kb)

            for u in proj_units(0):   # chunk 0 projection runs immediately
                u()
            nc.sync.dma_start(wd_sb[:], wd)  # dense-weight prefetch

            fin = None
            for j in range(NQC):
                if j + 1 < NSC:
                    load_chunk(j + 1)
                    fills_hi.extend(proj_units(j + 1))
                if j >= 1:
                    for st in range((j - 1) * 4, j * 4):
                        for oc in range(2):
                            fills_lo.append(
                                lambda st=st, oc=oc: emit_dense_oc(st, oc)
                            )
                # last j ends on an even head: its shorter den chain is the
                # only one fully exposed before the final dense blocks
                horder = (0, 1, 3, 2) if j == NQC - 1 else tuple(range(HPG))
                for hi_, h in enumerate(horder):
                    fin = emit_attn_head(h, j, fin, horder)
                while fills_hi:  # next j's scores depend on these
                    pop_fill(lo_ok=False)
            fin()
            while fills_lo:
                pop_fill()
        for st in range((NQC - 1) * 4, NKT):
            emit_dense_st(st)

    nc.compile()
    return nc


def _classify_mask(mask):
    """Classify 128(k) x 128(q) score blocks from the actual mask contents.

    Returns (ktiles, mk_arr):
      ktiles[j]: list of (t, lo, tri) per computed k-tile for q-chunk j:
        lo: first kept 128-col block within the 512-wide q-chunk (cols
            [0, lo*128) are fully masked and simply never computed/read)
        tri: [(col_block, uid), ...] 128-col blocks needing a factor mult
      mk_arr: [128, NU, 128] bf16 multiplicative factors exp(-1e9*m/8)
    """
    m2 = np.asarray(mask, dtype=np.float32).reshape(S, S)
    F = np.exp(m2 * np.float32(-1.25e8))  # exp(-1e9*m/8); 0/1 masks -> 0/1
    if (F.max(axis=1) == 0.0).any():
        raise RuntimeError("mask has fully-masked rows; unsupported")
    blocks = F.reshape(NKT, 128, NKT, 128)  # [qi, qr, t, kr]
    kept = (blocks == 1.0).all(axis=(1, 3))  # [qi, t]
    skip = (blocks == 0.0).all(axis=(1, 3))

    NB = QC // 128  # 128-col blocks per q-chunk
    ktiles = []
    uniq = {}
    mk_tiles = []

    def factor_uid(qi, t):
        fb = np.ascontiguousarray(
            F[qi * 128 : (qi + 1) * 128, t * KT : (t + 1) * KT].T
        ).astype(NPBF16)
        key = fb.tobytes()
        if key not in uniq:
            uniq[key] = len(mk_tiles)
            mk_tiles.append(fb)
        return uniq[key]

    for j in range(NQC):
        qis = list(range(j * NB, (j + 1) * NB))
        tl = []
        for t in range(NKT):
            stats = [
                "k" if kept[qi, t] else ("s" if skip[qi, t] else "m")
                for qi in qis
            ]
            if all(s == "s" for s in stats):
                continue
            lo = next(i for i, s in enumerate(stats) if s != "s")
            tri = []
            for i in range(lo, NB):
                if stats[i] == "k":
                    continue
                # mixed OR interior skip (multiply by its factor / zeros)
                tri.append((i, factor_uid(qis[i], t)))
            tl.append((t, lo, tri))
        if not tl:
            raise RuntimeError("q-chunk with no kept k-tiles; unsupported")
        # the first computed tile must span the full chunk (av 'start' MM)
        if tl[0][1] != 0:
            t0, _, tri0 = tl[0]
            tri0 = [(i, u) for i, u in tri0]
            have = {i for i, _ in tri0}
            for i in range(tl[0][1]):
                if i not in have:
                    tri0.append((i, factor_uid(qis[i], t0)))
            tl[0] = (t0, 0, sorted(tri0))
        ktiles.append(tl)
    if not mk_tiles:
        mk_tiles.append(np.ones((128, KT), dtype=NPBF16))
    mk_arr = np.ascontiguousarray(np.stack(mk_tiles, axis=0).transpose(1, 0, 2))
    return ktiles, mk_arr


def _xt_prep(x):
    """[S, D] f32 -> [NSC, 128, DK, SC] bf16, d-major, contiguous S-quarters."""
    xt = x.T.astype(NPBF16)  # [D, S]
    a = xt.reshape(DK, 128, NSC, SC).transpose(2, 1, 0, 3)
    return np.ascontiguousarray(a)


def kernel(v, k, q, mask, wq_w, wq_b, wk_w, wk_b, wv_w, wv_b, dense_w, dense_b):
    global LAST_RESULT
    v = np.asarray(v, dtype=np.float32)
    k = np.asarray(k, dtype=np.float32)
    q = np.asarray(q, dtype=np.float32)
    mask = np.asarray(mask, dtype=np.float32)
    wq_w = np.asarray(wq_w, dtype=np.float32)
    wk_w = np.asarray(wk_w, dtype=np.float32)
    wv_w = np.asarray(wv_w, dtype=np.float32)
    dense_w = np.asarray(dense_w, dtype=np.float32)
    wq_b = np.asarray(wq_b, dtype=np.float32)
    wk_b = np.asarray(wk_b, dtype=np.float32)
    wv_b = np.asarray(wv_b, dtype=np.float32)
    dense_b = np.asarray(dense_b, dtype=np.float32)

    ktiles, mk_arr = _classify_mask(mask)
    zero_bias = not (np.any(wq_b) or np.any(wk_b))
    key = (
        tuple(tuple((t, lo, tuple(tri)) for t, lo, tri in tl) for tl in ktiles),
        mk_arr.shape[1],
        zero_bias,
        "v2",
    )
    if key not in _CACHE:
        _CACHE[key] = _build(ktiles, mk_arr.shape[1], zero_bias)
    nc = _CACHE[key]

    # per-batch inputs (shared by the 4 cores of each batch)
    xq_b = [_xt_prep(q[b]) for b in range(B)]
    xk_b = [_xt_prep(k[b]) for b in range(B)]
    xv_b = [_xt_prep(v[b]) for b in range(B)]

    # per-group weights
    def wslice(w, g):
        ws = w[g * OG : (g + 1) * OG, :].T.astype(NPBF16)  # [D, OG]
        return np.ascontiguousarray(ws.reshape(DK, 128, OG).transpose(1, 0, 2))

    def bslice(b_, g):
        return np.ascontiguousarray(
            b_[g * OG : (g + 1) * OG].astype(np.float32).reshape(2, 128).T
        )

    wq_g = [wslice(wq_w, g) for g in range(GROUPS)]
    wk_g = [wslice(wk_w, g) for g in range(GROUPS)]
    wv_g = [wslice(wv_w, g) for g in range(GROUPS)]
    qb_g = [bslice(wq_b, g) for g in range(GROUPS)]
    kb_g = [bslice(wk_b, g) for g in range(GROUPS)]
    wd_g = []
    for g in range(GROUPS):
        ds = dense_w[:, g * OG : (g + 1) * OG].T.astype(NPBF16)  # [OG, D]
        wd_g.append(np.ascontiguousarray(ds.reshape(2, 128, D_MODEL).transpose(1, 0, 2)))

    in_maps = []
    for c in range(N_CORES):
        b, g = c // GROUPS, c % GROUPS
        in_maps.append(
            {
                "xq": xq_b[b],
                "xk": xk_b[b],
                "xv": xv_b[b],
                "wq": wq_g[g],
                "wk": wk_g[g],
                "wv": wv_g[g],
                "wd": wd_g[g],
                "qb": qb_g[g],
                "kb": kb_g[g],
                "mk": mk_arr,
            }
        )

    kw = dict(trace=True, **TRACE_KW) if TRACE else {}
    res = run_bass_kernel_spmd(nc, in_maps, core_ids=list(range(N_CORES)), **kw)
    LAST_RESULT = res

    corr = dense_w @ wv_b + dense_b  # v-bias pushed through dense, + dense bias
    out = np.empty((B, S, D_MODEL), dtype=np.float32)
    for b in range(B):
        acc = np.zeros((S, D_MODEL), dtype=np.float32)
        for g in range(GROUPS):
            acc += np.asarray(res.results[b * GROUPS + g]["outp"], dtype=np.float32)
        out[b] = acc + corr
    return out
